# revision 1
# baseline (speedup 1.0000x reference)
"""Trainium2 Bass kernel for nn_LinearPerBlockQuant (per-block fake-quant linear).

  out = fake_quant(x; a_scales, a_zeros) @ fake_quant(W; w_scales, w_zeros).T + bias

Shapes: x (1024, 4096) f32, W (4096, 4096), block size 4 along IN,
w_scales/w_zeros (4096, 1024), a_scales/a_zeros (1024,), bias (4096,).

Sharding: column-parallel over 8 NeuronCores — each core owns 512 output
features (W rows, scales, bias shards); x is replicated. Host concatenates
the 8 (512, 1024) partial outputs and transposes.

Device-side per core:
  - x arrives pre-transposed+block-permuted  xT[r*1024+kb, b] = x[b, 4*kb+r]
    so per-k quant scales are per-partition scalars (ACT scale/bias fusion),
    and W arrives column-permuted the same way so per-block weight scales are
    dense (128, 1024) tensor_tensor operands.
  - quant: q = sat_u8(v * (1/s) + z)  (HW u8 conversion = round-half-even +
    saturate, verified on HW == clip(round(.), 0, 255))
  - weight dequant in natural layout, PE-transposed to (K, O) tiles, drained
    to SBUF as float32r (FP22-rounded) for full-rate fp32r matmuls.
  - matmul: psum(o, b) += wqT.T @ xqT over 32 K-strips; ACT adds bias on drain.
  - activation dequant (x2) double-buffers its output (qx/qx2) across b-chunks
    so it overlaps the previous chunk's matmuls; qx2 reuses the weight-scratch
    SBUF zone released after the W phase.
"""
import numpy as np
from contextlib import ExitStack

import concourse.bass as bass
import concourse.tile as tile
from concourse import bacc, mybir
from concourse.bass_utils import run_bass_kernel_spmd
from concourse.masks import make_identity

F32 = mybir.dt.float32
F32R = mybir.dt.float32r
U8 = mybir.dt.uint8
OP = mybir.AluOpType
AF = mybir.ActivationFunctionType

B, IN, OUT, BS = 1024, 4096, 4096, 4
NCORES = 8
OSH = OUT // NCORES          # 512 out-features per core
NB = IN // BS                # 1024 blocks along IN
NKT = IN // 128              # 32 k-strips of 128
NBC = 4                      # b-chunks for the matmul moving dim
BC = B // NBC                # 256 (>=256 keeps fp32r matmul at 1 cycle/row)
NOT = OSH // 128             # 4 output-feature tiles per core

_CACHE = {}


def _build_nc():
    nc = bacc.Bacc("TRN2", target_bir_lowering=False, debug=False)

    xT_d = nc.dram_tensor("xT", [IN, B], F32, kind="ExternalInput").ap()
    w_d = nc.dram_tensor("w", [OSH, IN], F32, kind="ExternalInput").ap()
    ws_d = nc.dram_tensor("ws", [OSH, NB], F32, kind="ExternalInput").ap()
    wz_d = nc.dram_tensor("wz", [OSH, NB], F32, kind="ExternalInput").ap()
    asc_d = nc.dram_tensor("asc", [128, NKT // 4], F32, kind="ExternalInput").ap()
    az_d = nc.dram_tensor("az", [128, NKT // 4], F32, kind="ExternalInput").ap()
    bias_d = nc.dram_tensor("bias", [128, NOT], F32, kind="ExternalInput").ap()
    out_d = nc.dram_tensor("out", [OSH, B], F32, kind="ExternalOutput").ap()

    with tile.TileContext(nc) as tc, ExitStack() as ctx:
        const = ctx.enter_context(tc.tile_pool(name="const", bufs=1))
        big = ctx.enter_context(tc.tile_pool(name="big", bufs=1))
        xrp = ctx.enter_context(tc.tile_pool(name="xr", bufs=2))
        outp = ctx.enter_context(tc.tile_pool(name="outp", bufs=3))
        pst = ctx.enter_context(tc.tile_pool(name="pst", bufs=2, space="PSUM"))
        psm = ctx.enter_context(tc.tile_pool(name="psm", bufs=4, space="PSUM"))

        # ---- constants / small tensors ----
        asc_t = const.tile([128, NKT // 4], F32)
        nc.sync.dma_start(asc_t[:], asc_d)
        az_t = const.tile([128, NKT // 4], F32)
        nc.sync.dma_start(az_t[:], az_d)
        bias_t = const.tile([128, NOT], F32)
        nc.sync.dma_start(bias_t[:], bias_d)
        ras_t = const.tile([128, NKT // 4], F32)
        nc.vector.reciprocal(ras_t[:], asc_t[:])
        # nzsa = -(za * sa)
        nzsa_t = const.tile([128, NKT // 4], F32)
        nc.vector.scalar_tensor_tensor(nzsa_t[:], az_t[:], -1.0, asc_t[:],
                                       OP.mult, OP.mult)
        ident = const.tile([128, 128], F32)
        make_identity(nc, ident[:])

        # ---- resident big tensors ----
        # q8: quantized activations, u8, strip kt at cols [kt*1024, +1024)
        q8_t = big.tile([128, NKT * B], U8)
        # qx: dequantized activations (f32 bits, fp32r-rounded), one b-chunk
        qx_t = big.tile([128, NKT * BC], F32)
        # wqT: dequantized transposed weights, strip kt at cols [kt*OSH, +OSH)
        wqT_t = big.tile([128, NKT * OSH], F32)

        # ---- activations: DMA + quantize (per k-strip) ----
        def emit_hoisted_x():
            for kt in range(NKT):
                xr = xrp.tile([128, B], F32, tag="xr")
                nc.sync.dma_start(xr[:], xT_d[128 * kt:128 * (kt + 1), :])
                c = kt % (NKT // 4)
                # q = sat_u8(x*(1/sa) + za) == clip(round(x/sa + za), 0, 255)
                nc.scalar.activation(q8_t[:, kt * B:(kt + 1) * B], xr[:],
                                     AF.Identity,
                                     bias=az_t[:, c:c + 1],
                                     scale=ras_t[:, c:c + 1])

        # ---- interleaved weight-prep / activation-dequant / matmul ----
        # Emission order pipelines: W(ot) chains feed MM(bc0, ot) immediately;
        # x2 conversions for later b-chunks slot into DVE gaps; the qx/qx2
        # ping-pong removes write-after-read stalls between b-chunks.
        NC8 = NKT // 4
        q8_v = q8_t[:].rearrange("p (kt b) -> p kt b", b=B)

        def emit_x2(bc, qx):
            qx_v = qx[:].rearrange("p (kt c) -> p kt c", c=BC)
            for kt in range(NKT // 2):  # pairs (kt, kt+8) share scale col
                c = kt % NC8
                base = (kt // NC8) * (2 * NC8) + c
                src = q8_v[:, base:base + NC8 + 1:NC8, bc * BC:(bc + 1) * BC]
                dst = qx_v[:, base:base + NC8 + 1:NC8, :]
                nc.vector.tensor_scalar(dst.bitcast(F32R), src,
                                        asc_t[:, c:c + 1], nzsa_t[:, c:c + 1],
                                        OP.mult, OP.add)

        def emit_mm(bc, ot, qx):
            pacc = psm.tile([128, BC], F32)
            for kt in range(NKT):
                lhsT = wqT_t[:, kt * OSH + 128 * ot: kt * OSH + 128 * (ot + 1)]
                rhs = qx[:, kt * BC:(kt + 1) * BC]
                nc.tensor.matmul(pacc[:], lhsT.bitcast(F32R),
                                 rhs.bitcast(F32R),
                                 start=(kt == 0), stop=(kt == NKT - 1))
            ob = outp.tile([128, BC], F32)
            nc.scalar.activation(ob[:], pacc[:], AF.Identity,
                                 bias=bias_t[:, ot:ot + 1], scale=1.0)
            nc.sync.dma_start(
                out_d[128 * ot:128 * (ot + 1), bc * BC:(bc + 1) * BC], ob[:])

        def emit_w_scales(ot, wsp, wzp, rwsp):
            ws_t = wsp.tile([128, NB], F32, tag="ws")
            nc.sync.dma_start(ws_t[:], ws_d[128 * ot:128 * (ot + 1), :])
            wz_t = wzp.tile([128, NB], F32, tag="wz")
            nc.sync.dma_start(wz_t[:], wz_d[128 * ot:128 * (ot + 1), :])
            rws_t = rwsp.tile([128, NB], F32, tag="rws")
            nc.vector.reciprocal_approx_fast(rws_t[:], ws_t[:])
            return ws_t, wz_t, rws_t

        def emit_w(ot, scales, wrp, tdp, q8wp, wqnp, wr0=None):
            ws_t, wz_t, rws_t = scales
            for rc in range(IN // NB):  # 4 chunks of (128, 1024)
                if rc == 0 and wr0 is not None:
                    wr = wr0
                else:
                    wr = wrp.tile([128, NB], F32, tag="wr")
                    nc.sync.dma_start(
                        wr[:],
                        w_d[128 * ot:128 * (ot + 1), NB * rc:NB * (rc + 1)])
                t_t = tdp.tile([128, NB], F32)
                nc.gpsimd.tensor_tensor(t_t[:], wr[:], rws_t[:], OP.mult)
                q8w = q8wp.tile([128, NB], U8)
                nc.vector.tensor_tensor(q8w[:], t_t[:], wz_t[:], OP.add)
                d_t = tdp.tile([128, NB], F32)
                nc.vector.tensor_tensor(d_t[:], q8w[:], wz_t[:], OP.subtract)
                wqn = wqnp.tile([128, NB], F32)
                if (ot * 4 + rc) % 2 == 0:
                    nc.gpsimd.tensor_tensor(wqn[:], d_t[:], ws_t[:], OP.mult)
                else:
                    nc.vector.tensor_tensor(wqn[:], d_t[:], ws_t[:], OP.mult)
                # transpose 8 (128,128) blocks -> wqT strips (f32r)
                for half in range(2):
                    ps = pst.tile([128, 512], F32)
                    for j2 in range(4):
                        j = half * 4 + j2
                        nc.tensor.transpose(ps[:, 128 * j2:128 * (j2 + 1)],
                                            wqn[:, 128 * j:128 * (j + 1)],
                                            ident[:])
                    kt0 = 8 * rc + 4 * half
                    dst = wqT_t[:].rearrange("p (kt o) -> p kt o", o=OSH)[
                        :, kt0:kt0 + 4, 128 * ot:128 * (ot + 1)]
                    nc.vector.tensor_copy(dst.bitcast(F32R), ps[:])

        with tc.tile_pool(name="wr", bufs=2) as wrp, \
             tc.tile_pool(name="wsp", bufs=2) as wsp, \
             tc.tile_pool(name="wzp", bufs=2) as wzp, \
             tc.tile_pool(name="rws", bufs=2) as rwsp, \
             tc.tile_pool(name="td", bufs=3) as tdp, \
             tc.tile_pool(name="q8w", bufs=2) as q8wp, \
             tc.tile_pool(name="wqn", bufs=1) as wqnp:
            # hoisted: ot0 scales + first w chunk land before the x-DMA flood
            sc0 = emit_w_scales(0, wsp, wzp, rwsp)
            wr0 = wrp.tile([128, NB], F32, tag="wr")
            nc.sync.dma_start(wr0[:], w_d[0:128, 0:NB])
            emit_hoisted_x()
            for ot in range(NOT):
                sc = sc0 if ot == 0 else emit_w_scales(ot, wsp, wzp, rwsp)
                emit_w(ot, sc, wrp, tdp, q8wp, wqnp,
                       wr0=wr0 if ot == 0 else None)
            emit_x2(0, qx_t)
            for ot in range(NOT):
                emit_mm(0, ot, qx_t)

        # W scratch released; qx2 reuses the zone for bc ping-pong.
        with tc.tile_pool(name="big2", bufs=1) as big2:
            qx2_t = big2.tile([128, NKT * BC], F32)
            emit_x2(1, qx2_t)
            for ot in range(NOT):
                emit_mm(1, ot, qx2_t)
            emit_x2(2, qx_t)
            for ot in range(NOT):
                emit_mm(2, ot, qx_t)
            emit_x2(3, qx2_t)
            for ot in range(NOT):
                emit_mm(3, ot, qx2_t)

    nc.compile()
    return nc


def _get_nc():
    if "nc" not in _CACHE:
        _CACHE["nc"] = _build_nc()
    return _CACHE["nc"]


def _prep_inputs(x, weight, bias, w_scales, w_zeros, a_scales, a_zeros):
    """Host-side shard/layout prep. Pure slicing/permutation, no arithmetic."""
    x = np.ascontiguousarray(x, np.float32)
    # xT[r*NB + kb, b] = x[b, kb*BS + r]
    xT = np.ascontiguousarray(
        x.reshape(B, NB, BS).transpose(2, 1, 0).reshape(IN, B))
    asc2 = np.ascontiguousarray(
        np.asarray(a_scales, np.float32).reshape(NKT // 4, 128).T)
    az2 = np.ascontiguousarray(
        np.asarray(a_zeros, np.float32).reshape(NKT // 4, 128).T)
    in_maps = []
    for c in range(NCORES):
        sl = slice(c * OSH, (c + 1) * OSH)
        wsh = np.asarray(weight[sl], np.float32)
        wperm = np.ascontiguousarray(
            wsh.reshape(OSH, NB, BS).transpose(0, 2, 1).reshape(OSH, IN))
        in_maps.append({
            "xT": xT,
            "w": wperm,
            "ws": np.ascontiguousarray(np.asarray(w_scales[sl], np.float32)),
            "wz": np.ascontiguousarray(np.asarray(w_zeros[sl], np.float32)),
            "asc": asc2,
            "az": az2,
            "bias": np.ascontiguousarray(
                np.asarray(bias[sl], np.float32).reshape(NOT, 128).T),
        })
    return in_maps


def kernel(x, weight, bias, w_scales, w_zeros, a_scales, a_zeros, _res_out=None):
    nc = _get_nc()
    in_maps = _prep_inputs(x, weight, bias, w_scales, w_zeros, a_scales, a_zeros)
    res = run_bass_kernel_spmd(nc, in_maps, core_ids=list(range(NCORES)))
    if _res_out is not None:
        _res_out.append(res)
    outT = np.concatenate([res.results[c]["out"] for c in range(NCORES)], axis=0)
    return np.ascontiguousarray(outT.T)



# revision 6
# speedup vs baseline: 1.5611x; 1.5611x over previous
"""Trainium2 Bass kernel for nn_LinearPerBlockQuant (per-block fake-quant linear).

  out = fake_quant(x; a_scales, a_zeros) @ fake_quant(W; w_scales, w_zeros).T + bias

Shapes: x (1024, 4096) f32, W (4096, 4096), block size 4 along IN,
w_scales/w_zeros (4096, 1024), a_scales/a_zeros (1024,), bias (4096,).

Sharding: column-parallel over 8 NeuronCores -- each core owns 512 output
features (W rows, scales, bias shards); x is replicated. Host concatenates
the 8 (512, 1024) partial outputs and transposes.

Device-side per core (strip-streaming design):
  - x and W both arrive pre-transposed + block-permuted on the k axis:
      xT[r*1024+kb, b] = x[b, 4*kb+r];  wT[r*1024+kb, o] = W[o, 4*kb+r]
    so k is the partition dim everywhere and per-k activation quant
    scales are per-partition scalars (ACT scale/bias fusion). Weight
    scales arrive transposed (wsT/wzT (1024, 512)), so in a k-strip the
    per-(o, block) scales are dense (128, 512) tensor operands shared by
    the 4 strips of one kb-octave.
  - quant: q = sat_u8(v * (1/s) + z)  (HW u8 conversion = round-half-even +
    saturate == clip(round(.), 0, 255), HW-verified)
  - x path on ACT: f32 -> u8 (quant), u8 -> bf16 (dequant), both with
    per-partition scale/bias fusion. W path: gpsimd mult + DVE add/sub/mult,
    result bf16. bf16 keeps full qx (64KB/part) + wqT (32KB/part) resident.
  - matmul: 8 psum tiles (128, 512) = all 8 banks, one accumulation chain
    per (ot, b-half), accumulated strip-by-strip as data lands (bf16 =
    1 cycle/row). A dummy transpose holds PE back until strip DUMMY_STRIP
    is ready so the matmul stream runs gapless at full p-state.
  - drain: ACT adds bias on psum drain; one output DMA per 128-row tile.
"""
import numpy as np
from contextlib import ExitStack

import concourse.bass as bass
import concourse.tile as tile
from concourse import bacc, mybir
from concourse.bass_utils import run_bass_kernel_spmd
from concourse.masks import make_identity

F32 = mybir.dt.float32
BF16 = mybir.dt.bfloat16
U8 = mybir.dt.uint8
OP = mybir.AluOpType
AF = mybir.ActivationFunctionType

B, IN, OUT, BS = 1024, 4096, 4096, 4
NCORES = 8
OSH = OUT // NCORES          # 512 out-features per core
NB = IN // BS                # 1024 blocks along IN
NKT = IN // 128              # 32 k-strips of 128
NOCT = 8                     # kb-octaves (128 kb values each)
NOT = OSH // 128             # 4 output-feature tiles per core
OUT_BF16 = False             # write output as bf16 (halves output DMA)
DUMMY_STRIP = None           # hold PE until this strip's qx is ready

_CACHE = {}


def _build_nc():
    nc = bacc.Bacc("TRN2", target_bir_lowering=False, debug=False)

    xT_d = nc.dram_tensor("xT", [IN, B], F32, kind="ExternalInput").ap()
    wT_d = nc.dram_tensor("wT", [IN, OSH], F32, kind="ExternalInput").ap()
    wsT_d = nc.dram_tensor("wsT", [NB, OSH], F32, kind="ExternalInput").ap()
    wzT_d = nc.dram_tensor("wzT", [NB, OSH], F32, kind="ExternalInput").ap()
    asc_d = nc.dram_tensor("asc", [128, NOCT], F32, kind="ExternalInput").ap()
    az_d = nc.dram_tensor("az", [128, NOCT], F32, kind="ExternalInput").ap()
    bias_d = nc.dram_tensor("bias", [128, NOT], F32, kind="ExternalInput").ap()
    out_dt = BF16 if OUT_BF16 else F32
    out_d = nc.dram_tensor("out", [OSH, B], out_dt, kind="ExternalOutput").ap()

    with tile.TileContext(nc) as tc, ExitStack() as ctx:
        const = ctx.enter_context(tc.tile_pool(name="const", bufs=1))
        big = ctx.enter_context(tc.tile_pool(name="big", bufs=1))
        xrp = ctx.enter_context(tc.tile_pool(name="xr", bufs=5))
        q8p = ctx.enter_context(tc.tile_pool(name="q8", bufs=3))
        wtp = ctx.enter_context(tc.tile_pool(name="wt", bufs=4))
        wsp = ctx.enter_context(tc.tile_pool(name="wsp", bufs=3))
        wzp = ctx.enter_context(tc.tile_pool(name="wzp", bufs=3))
        rwsp = ctx.enter_context(tc.tile_pool(name="rws", bufs=3))
        tdp = ctx.enter_context(tc.tile_pool(name="td", bufs=4))
        q8wp = ctx.enter_context(tc.tile_pool(name="q8w", bufs=3))
        outp = ctx.enter_context(tc.tile_pool(name="outp", bufs=2))
        psm = ctx.enter_context(tc.tile_pool(name="psm", bufs=1, space="PSUM"))

        # ---- constants / small tensors ----
        asc_t = const.tile([128, NOCT], F32)
        nc.sync.dma_start(asc_t[:], asc_d)
        az_t = const.tile([128, NOCT], F32)
        nc.sync.dma_start(az_t[:], az_d)
        bias_t = const.tile([128, NOT], F32)
        nc.sync.dma_start(bias_t[:], bias_d)
        ras_t = const.tile([128, NOCT], F32)
        nc.vector.reciprocal(ras_t[:], asc_t[:])
        # nzsa = -(za * sa)
        nzsa_t = const.tile([128, NOCT], F32)
        nc.vector.scalar_tensor_tensor(nzsa_t[:], az_t[:], -1.0, asc_t[:],
                                       OP.mult, OP.mult)
        ident = None
        if DUMMY_STRIP is not None:
            ident = const.tile([128, 128], F32)
            make_identity(nc, ident[:])

        # ---- resident big tensors ----
        qx_t = big.tile([128, NKT * B], BF16)     # dequant activations
        wq_t = big.tile([128, NKT * OSH], BF16)   # dequant transposed weights

        # 8 psum accumulators: (ot, b-half), each (128, 512) = one bank
        pacc = [psm.tile([128, 512], F32, name=f"pacc{j}") for j in range(8)]

        dummy_emitted = [DUMMY_STRIP is None]

        def emit_strip(i, oct_, r, scales):
            kt = r * NOCT + oct_
            ws_t, wz_t, rws_t = scales
            # --- DMAs (w first: its chain is one hop longer) ---
            wt_i = wtp.tile([128, OSH], F32, tag="wt")
            nc.sync.dma_start(wt_i[:], wT_d[128 * kt:128 * (kt + 1), :])
            xr_i = xrp.tile([128, B], F32, tag="xr")
            nc.sync.dma_start(xr_i[:], xT_d[128 * kt:128 * (kt + 1), :])
            # --- W chain: t = w*rws (Pool); q8w = u8(t+wz); d = q8w-wz;
            #     wq = bf16(d*ws) ---
            t_t = tdp.tile([128, OSH], F32, tag="t")
            nc.gpsimd.tensor_tensor(t_t[:], wt_i[:], rws_t[:], OP.mult)
            q8w = q8wp.tile([128, OSH], U8, tag="q8w")
            nc.vector.tensor_tensor(q8w[:], t_t[:], wz_t[:], OP.add)
            d_t = tdp.tile([128, OSH], F32, tag="d")
            nc.vector.tensor_tensor(d_t[:], q8w[:], wz_t[:], OP.subtract)
            wq_v = wq_t[:, kt * OSH:(kt + 1) * OSH]
            nc.vector.tensor_tensor(wq_v, d_t[:], ws_t[:], OP.mult)
            # --- x chain on ACT: q8 = u8(x*(1/sa)+za); qx = bf16(q8*sa-za*sa)
            q8_i = q8p.tile([128, B], U8, tag="q8")
            nc.scalar.activation(q8_i[:], xr_i[:], AF.Identity,
                                 bias=az_t[:, oct_:oct_ + 1],
                                 scale=ras_t[:, oct_:oct_ + 1])
            qx_v = qx_t[:, kt * B:(kt + 1) * B]
            nc.scalar.activation(qx_v, q8_i[:], AF.Identity,
                                 bias=nzsa_t[:, oct_:oct_ + 1],
                                 scale=asc_t[:, oct_:oct_ + 1])

        def emit_mms(i, kt):
            if not dummy_emitted[0]:
                dk = DUMMY_STRIP
                dkt = (dk % 4) * NOCT + dk // 4
                nc.tensor.transpose(pacc[0][:, 0:128],
                                    qx_t[:, dkt * B:dkt * B + 128],
                                    ident[:])
                dummy_emitted[0] = True
            for ot in range(NOT):
                lhsT = wq_t[:, kt * OSH + 128 * ot:kt * OSH + 128 * (ot + 1)]
                for b2 in range(2):
                    rhs = qx_t[:, kt * B + 512 * b2:kt * B + 512 * (b2 + 1)]
                    nc.tensor.matmul(pacc[ot * 2 + b2][:], lhsT, rhs,
                                     start=(i == 0), stop=(i == NKT - 1))

        for oct_ in range(NOCT):
            ws_t = wsp.tile([128, OSH], F32, tag="ws")
            nc.sync.dma_start(ws_t[:], wsT_d[128 * oct_:128 * (oct_ + 1), :])
            wz_t = wzp.tile([128, OSH], F32, tag="wz")
            nc.sync.dma_start(wz_t[:], wzT_d[128 * oct_:128 * (oct_ + 1), :])
            rws_t = rwsp.tile([128, OSH], F32, tag="rws")
            nc.vector.reciprocal_approx_fast(rws_t[:], ws_t[:])
            scales = (ws_t, wz_t, rws_t)
            for r in range(4):
                i = oct_ * 4 + r
                emit_strip(i, oct_, r, scales)
                if DUMMY_STRIP is None or i >= DUMMY_STRIP:
                    for j in (range(i + 1) if (DUMMY_STRIP is not None
                                               and i == DUMMY_STRIP)
                              else (i,)):
                        emit_mms(j, (j % 4) * NOCT + j // 4)

        # ---- drain: bias add on ACT, one DMA per 128-row out tile ----
        for ot in range(NOT):
            ob = outp.tile([128, B], out_dt, tag="ob")
            for b2 in range(2):
                nc.scalar.activation(ob[:, 512 * b2:512 * (b2 + 1)],
                                     pacc[ot * 2 + b2][:], AF.Identity,
                                     bias=bias_t[:, ot:ot + 1], scale=1.0)
            nc.sync.dma_start(out_d[128 * ot:128 * (ot + 1), :], ob[:])

    nc.compile()
    return nc


def _get_nc():
    if "nc" not in _CACHE:
        _CACHE["nc"] = _build_nc()
    return _CACHE["nc"]


def _prep_inputs(x, weight, bias, w_scales, w_zeros, a_scales, a_zeros):
    """Host-side shard/layout prep. Pure slicing/permutation, no arithmetic."""
    x = np.ascontiguousarray(x, np.float32)
    # xT[r*NB + kb, b] = x[b, kb*BS + r]
    xT = np.ascontiguousarray(
        x.reshape(B, NB, BS).transpose(2, 1, 0).reshape(IN, B))
    asc2 = np.ascontiguousarray(
        np.asarray(a_scales, np.float32).reshape(NOCT, 128).T)
    az2 = np.ascontiguousarray(
        np.asarray(a_zeros, np.float32).reshape(NOCT, 128).T)
    in_maps = []
    for c in range(NCORES):
        sl = slice(c * OSH, (c + 1) * OSH)
        wsh = np.asarray(weight[sl], np.float32)
        # wT[r*NB + kb, o] = W[o, kb*BS + r]
        wT = np.ascontiguousarray(
            wsh.reshape(OSH, NB, BS).transpose(2, 1, 0).reshape(IN, OSH))
        in_maps.append({
            "xT": xT,
            "wT": wT,
            "wsT": np.ascontiguousarray(
                np.asarray(w_scales[sl], np.float32).T),
            "wzT": np.ascontiguousarray(
                np.asarray(w_zeros[sl], np.float32).T),
            "asc": asc2,
            "az": az2,
            "bias": np.ascontiguousarray(
                np.asarray(bias[sl], np.float32).reshape(NOT, 128).T),
        })
    return in_maps


def kernel(x, weight, bias, w_scales, w_zeros, a_scales, a_zeros, _res_out=None):
    nc = _get_nc()
    in_maps = _prep_inputs(x, weight, bias, w_scales, w_zeros, a_scales, a_zeros)
    res = run_bass_kernel_spmd(nc, in_maps, core_ids=list(range(NCORES)))
    if _res_out is not None:
        _res_out.append(res)
    outT = np.concatenate([np.asarray(res.results[c]["out"], np.float32)
                           for c in range(NCORES)], axis=0)
    return np.ascontiguousarray(outT.T)


# revision 7
# speedup vs baseline: 1.6296x; 1.0439x over previous
"""Trainium2 Bass kernel for nn_LinearPerBlockQuant (per-block fake-quant linear).

  out = fake_quant(x; a_scales, a_zeros) @ fake_quant(W; w_scales, w_zeros).T + bias

Shapes: x (1024, 4096) f32, W (4096, 4096), block size 4 along IN,
w_scales/w_zeros (4096, 1024), a_scales/a_zeros (1024,), bias (4096,).

Sharding: column-parallel over 8 NeuronCores -- each core owns 512 output
features (W rows, scales, bias shards); x is replicated. Host concatenates
the 8 (512, 1024) partial outputs and transposes.

Device-side per core (strip-streaming design):
  - x and W both arrive pre-transposed + block-permuted on the k axis:
      xT[r*1024+kb, b] = x[b, 4*kb+r];  wT[r*1024+kb, o] = W[o, 4*kb+r]
    so k is the partition dim everywhere and per-k activation quant
    scales are per-partition scalars (ACT scale/bias fusion). Weight
    scales arrive transposed (wsT/wzT (1024, 512)), so in a k-strip the
    per-(o, block) scales are dense (128, 512) tensor operands shared by
    the 4 strips of one kb-octave (prefetched one octave ahead).
  - quant: q = sat_u8(v * (1/s) + z)  (HW u8 conversion = round-half-even +
    saturate == clip(round(.), 0, 255), HW-verified)
  - x path on ACT: f32 -> u8 (quant), u8 -> bf16 (dequant), both with
    per-partition scale/bias fusion. W path: gpsimd mult + DVE add/sub/mult,
    result bf16. bf16 keeps full qx (64KB/part) + wqT (32KB/part) resident.
  - matmul: 8 psum tiles (128, 512) = all 8 banks, one accumulation chain
    per (ot, b-half), accumulated strip-by-strip as data lands (bf16 =
    1 cycle/row).
  - drain: bias added on psum drain, alternating ACT/DVE so the 8 drains
    run in parallel pairs; output written bf16 (halves output DMA), one
    DMA per (ot, b-half) fired straight after its drain.
"""
import numpy as np
from contextlib import ExitStack

import concourse.bass as bass
import concourse.tile as tile
from concourse import bacc, mybir
from concourse.bass_utils import run_bass_kernel_spmd
from concourse.masks import make_identity

F32 = mybir.dt.float32
BF16 = mybir.dt.bfloat16
U8 = mybir.dt.uint8
OP = mybir.AluOpType
AF = mybir.ActivationFunctionType

B, IN, OUT, BS = 1024, 4096, 4096, 4
NCORES = 8
OSH = OUT // NCORES          # 512 out-features per core
NB = IN // BS                # 1024 blocks along IN
NKT = IN // 128              # 32 k-strips of 128
NOCT = 8                     # kb-octaves (128 kb values each)
NOT = OSH // 128             # 4 output-feature tiles per core
OUT_BF16 = True              # write output as bf16 (halves output DMA)
DUMMY_STRIP = None           # hold PE until this strip's qx is ready
NCST = 2 * NOCT + NOT        # asc | az | bias columns

_CACHE = {}


def _build_nc():
    nc = bacc.Bacc("TRN2", target_bir_lowering=False, debug=False)

    xT_d = nc.dram_tensor("xT", [IN, B], F32, kind="ExternalInput").ap()
    wT_d = nc.dram_tensor("wT", [IN, OSH], F32, kind="ExternalInput").ap()
    wsT_d = nc.dram_tensor("wsT", [NB, OSH], F32, kind="ExternalInput").ap()
    wzT_d = nc.dram_tensor("wzT", [NB, OSH], F32, kind="ExternalInput").ap()
    cst_d = nc.dram_tensor("cst", [128, NCST], F32, kind="ExternalInput").ap()
    out_dt = BF16 if OUT_BF16 else F32
    out_d = nc.dram_tensor("out", [OSH, B], out_dt, kind="ExternalOutput").ap()

    with tile.TileContext(nc) as tc, ExitStack() as ctx:
        const = ctx.enter_context(tc.tile_pool(name="const", bufs=1))
        big = ctx.enter_context(tc.tile_pool(name="big", bufs=1))
        xrp = ctx.enter_context(tc.tile_pool(name="xr", bufs=5))
        q8p = ctx.enter_context(tc.tile_pool(name="q8", bufs=3))
        wtp = ctx.enter_context(tc.tile_pool(name="wt", bufs=4))
        wsp = ctx.enter_context(tc.tile_pool(name="wsp", bufs=3))
        wzp = ctx.enter_context(tc.tile_pool(name="wzp", bufs=3))
        rwsp = ctx.enter_context(tc.tile_pool(name="rws", bufs=3))
        tdp = ctx.enter_context(tc.tile_pool(name="td", bufs=4))
        q8wp = ctx.enter_context(tc.tile_pool(name="q8w", bufs=3))
        outp = ctx.enter_context(tc.tile_pool(name="outp", bufs=8))
        psm = ctx.enter_context(tc.tile_pool(name="psm", bufs=1, space="PSUM"))

        # ---- constants: asc | az | bias in one DMA ----
        cst_t = const.tile([128, NCST], F32)
        nc.sync.dma_start(cst_t[:], cst_d)
        asc_t = cst_t[:, 0:NOCT]
        az_t = cst_t[:, NOCT:2 * NOCT]
        bias_t = cst_t[:, 2 * NOCT:]
        ras_t = const.tile([128, NOCT], F32)
        nc.vector.reciprocal(ras_t[:], asc_t)
        # nzsa = -(za * sa)
        nzsa_t = const.tile([128, NOCT], F32)
        nc.vector.scalar_tensor_tensor(nzsa_t[:], az_t, -1.0, asc_t,
                                       OP.mult, OP.mult)
        ident = None
        if DUMMY_STRIP is not None:
            ident = const.tile([128, 128], F32)
            make_identity(nc, ident[:])

        # ---- resident big tensors ----
        qx_t = big.tile([128, NKT * B], BF16)     # dequant activations
        wq_t = big.tile([128, NKT * OSH], BF16)   # dequant transposed weights

        # 8 psum accumulators: (ot, b-half), each (128, 512) = one bank
        pacc = [psm.tile([128, 512], F32, name=f"pacc{j}") for j in range(8)]

        dummy_emitted = [DUMMY_STRIP is None]

        def emit_scales(oct_):
            ws_t = wsp.tile([128, OSH], F32, tag="ws")
            nc.sync.dma_start(ws_t[:], wsT_d[128 * oct_:128 * (oct_ + 1), :])
            wz_t = wzp.tile([128, OSH], F32, tag="wz")
            nc.sync.dma_start(wz_t[:], wzT_d[128 * oct_:128 * (oct_ + 1), :])
            rws_t = rwsp.tile([128, OSH], F32, tag="rws")
            nc.vector.reciprocal_approx_fast(rws_t[:], ws_t[:])
            return ws_t, wz_t, rws_t

        def emit_strip(i, oct_, r, scales):
            kt = r * NOCT + oct_
            ws_t, wz_t, rws_t = scales
            # --- DMAs (w first: its chain is one hop longer) ---
            wt_i = wtp.tile([128, OSH], F32, tag="wt")
            nc.sync.dma_start(wt_i[:], wT_d[128 * kt:128 * (kt + 1), :])
            xr_i = xrp.tile([128, B], F32, tag="xr")
            nc.sync.dma_start(xr_i[:], xT_d[128 * kt:128 * (kt + 1), :])
            # --- W chain: t = w*rws (Pool); q8w = u8(t+wz); d = q8w-wz;
            #     wq = bf16(d*ws) ---
            t_t = tdp.tile([128, OSH], F32, tag="t")
            nc.gpsimd.tensor_tensor(t_t[:], wt_i[:], rws_t[:], OP.mult)
            q8w = q8wp.tile([128, OSH], U8, tag="q8w")
            nc.vector.tensor_tensor(q8w[:], t_t[:], wz_t[:], OP.add)
            d_t = tdp.tile([128, OSH], F32, tag="d")
            nc.vector.tensor_tensor(d_t[:], q8w[:], wz_t[:], OP.subtract)
            wq_v = wq_t[:, kt * OSH:(kt + 1) * OSH]
            nc.vector.tensor_tensor(wq_v, d_t[:], ws_t[:], OP.mult)
            # --- x chain on ACT: q8 = u8(x*(1/sa)+za); qx = bf16(q8*sa-za*sa)
            q8_i = q8p.tile([128, B], U8, tag="q8")
            nc.scalar.activation(q8_i[:], xr_i[:], AF.Identity,
                                 bias=az_t[:, oct_:oct_ + 1],
                                 scale=ras_t[:, oct_:oct_ + 1])
            qx_v = qx_t[:, kt * B:(kt + 1) * B]
            nc.scalar.activation(qx_v, q8_i[:], AF.Identity,
                                 bias=nzsa_t[:, oct_:oct_ + 1],
                                 scale=asc_t[:, oct_:oct_ + 1])

        def emit_mms(i, kt):
            if not dummy_emitted[0]:
                dk = DUMMY_STRIP
                dkt = (dk % 4) * NOCT + dk // 4
                nc.tensor.transpose(pacc[0][:, 0:128],
                                    qx_t[:, dkt * B:dkt * B + 128],
                                    ident[:])
                dummy_emitted[0] = True
            for ot in range(NOT):
                lhsT = wq_t[:, kt * OSH + 128 * ot:kt * OSH + 128 * (ot + 1)]
                for b2 in range(2):
                    rhs = qx_t[:, kt * B + 512 * b2:kt * B + 512 * (b2 + 1)]
                    nc.tensor.matmul(pacc[ot * 2 + b2][:], lhsT, rhs,
                                     start=(i == 0), stop=(i == NKT - 1))

        sc = emit_scales(0)
        for oct_ in range(NOCT):
            cur = sc
            for r in range(4):
                i = oct_ * 4 + r
                emit_strip(i, oct_, r, cur)
                if r == 0 and oct_ + 1 < NOCT:
                    sc = emit_scales(oct_ + 1)
                if DUMMY_STRIP is None or i >= DUMMY_STRIP:
                    for j in (range(i + 1) if (DUMMY_STRIP is not None
                                               and i == DUMMY_STRIP)
                              else (i,)):
                        emit_mms(j, (j % 4) * NOCT + j // 4)

        # ---- drain: bias add alternating ACT/DVE, out DMA per half ----
        for ot in range(NOT):
            for b2 in range(2):
                j = ot * 2 + b2
                ob = outp.tile([128, 512], out_dt, tag="ob")
                if j % 2 == 0:
                    nc.scalar.activation(ob[:], pacc[j][:], AF.Identity,
                                         bias=bias_t[:, ot:ot + 1], scale=1.0)
                else:
                    nc.vector.tensor_scalar(ob[:], pacc[j][:],
                                            bias_t[:, ot:ot + 1], None, OP.add)
                nc.sync.dma_start(
                    out_d[128 * ot:128 * (ot + 1), 512 * b2:512 * (b2 + 1)],
                    ob[:])

    nc.compile()
    return nc


def _get_nc():
    if "nc" not in _CACHE:
        _CACHE["nc"] = _build_nc()
    return _CACHE["nc"]


def _prep_inputs(x, weight, bias, w_scales, w_zeros, a_scales, a_zeros):
    """Host-side shard/layout prep. Pure slicing/permutation, no arithmetic."""
    x = np.ascontiguousarray(x, np.float32)
    # xT[r*NB + kb, b] = x[b, kb*BS + r]
    xT = np.ascontiguousarray(
        x.reshape(B, NB, BS).transpose(2, 1, 0).reshape(IN, B))
    asc2 = np.asarray(a_scales, np.float32).reshape(NOCT, 128).T
    az2 = np.asarray(a_zeros, np.float32).reshape(NOCT, 128).T
    in_maps = []
    for c in range(NCORES):
        sl = slice(c * OSH, (c + 1) * OSH)
        wsh = np.asarray(weight[sl], np.float32)
        # wT[r*NB + kb, o] = W[o, kb*BS + r]
        wT = np.ascontiguousarray(
            wsh.reshape(OSH, NB, BS).transpose(2, 1, 0).reshape(IN, OSH))
        cst = np.concatenate(
            [asc2, az2,
             np.asarray(bias[sl], np.float32).reshape(NOT, 128).T], axis=1)
        in_maps.append({
            "xT": xT,
            "wT": wT,
            "wsT": np.ascontiguousarray(
                np.asarray(w_scales[sl], np.float32).T),
            "wzT": np.ascontiguousarray(
                np.asarray(w_zeros[sl], np.float32).T),
            "cst": np.ascontiguousarray(cst),
        })
    return in_maps


def kernel(x, weight, bias, w_scales, w_zeros, a_scales, a_zeros, _res_out=None):
    nc = _get_nc()
    in_maps = _prep_inputs(x, weight, bias, w_scales, w_zeros, a_scales, a_zeros)
    res = run_bass_kernel_spmd(nc, in_maps, core_ids=list(range(NCORES)))
    if _res_out is not None:
        _res_out.append(res)
    outT = np.concatenate([np.asarray(res.results[c]["out"], np.float32)
                           for c in range(NCORES)], axis=0)
    return np.ascontiguousarray(outT.T)


# revision 13
# speedup vs baseline: 1.6453x; 1.0097x over previous
"""Trainium2 Bass kernel for nn_LinearPerBlockQuant (per-block fake-quant linear).

  out = fake_quant(x; a_scales, a_zeros) @ fake_quant(W; w_scales, w_zeros).T + bias

Shapes: x (1024, 4096) f32, W (4096, 4096), block size 4 along IN,
w_scales/w_zeros (4096, 1024), a_scales/a_zeros (1024,), bias (4096,).

Sharding: column-parallel over 8 NeuronCores -- each core owns 512 output
features (W rows, scales, bias shards); x is replicated. Host concatenates
the 8 (512, 1024) partial outputs and transposes.

Device-side per core (strip-streaming design):
  - x and W both arrive pre-transposed + block-permuted on the k axis:
      xT[r*1024+kb, b] = x[b, 4*kb+r];  wT[r*1024+kb, o] = W[o, 4*kb+r]
    so k is the partition dim everywhere and per-k activation quant
    scales are per-partition scalars (ACT scale/bias fusion). Weight
    scales arrive transposed (wsT/wzT (1024, 512)), so in a k-strip the
    per-(o, block) scales are dense (128, 512) tensor operands shared by
    the 4 strips of one kb-octave (prefetched one octave ahead).
  - quant: q = sat_u8(v * (1/s) + z)  (HW u8 conversion = round-half-even +
    saturate == clip(round(.), 0, 255), HW-verified)
  - x path on ACT: f32 -> u8 (quant), u8 -> bf16 (dequant), both with
    per-partition scale/bias fusion. W path: gpsimd mult + DVE add/sub/mult,
    result bf16. bf16 keeps full qx (64KB/part) + wqT (32KB/part) resident.
  - matmul: 8 psum tiles (128, 512) = all 8 banks, one accumulation chain
    per (ot, b-half), accumulated strip-by-strip as data lands (bf16 =
    1 cycle/row).
  - drain: bias added on psum drain, alternating ACT/DVE so the 8 drains
    run in parallel pairs; output written bf16 (halves output DMA), one
    DMA per (ot, b-half) fired straight after its drain.
"""
import numpy as np
from contextlib import ExitStack

import concourse.bass as bass
import concourse.tile as tile
from concourse import bacc, mybir
from concourse.bass_utils import run_bass_kernel_spmd
from concourse.masks import make_identity

F32 = mybir.dt.float32
BF16 = mybir.dt.bfloat16
U8 = mybir.dt.uint8
OP = mybir.AluOpType
AF = mybir.ActivationFunctionType

B, IN, OUT, BS = 1024, 4096, 4096, 4
NCORES = 8
OSH = OUT // NCORES          # 512 out-features per core
NB = IN // BS                # 1024 blocks along IN
NKT = IN // 128              # 32 k-strips of 128
NOCT = 8                     # kb-octaves (128 kb values each)
NOT = OSH // 128             # 4 output-feature tiles per core
OUT_BF16 = True              # write output as bf16 (halves output DMA)
DUMMY_STRIP = 11             # hold PE until this strip's qx is ready
NCST = 2 * NOCT + NOT        # asc | az | bias columns

_CACHE = {}


def _build_nc():
    nc = bacc.Bacc("TRN2", target_bir_lowering=False, debug=False)

    xT_d = nc.dram_tensor("xT", [IN, B], F32, kind="ExternalInput").ap()
    wT_d = nc.dram_tensor("wT", [IN, OSH], F32, kind="ExternalInput").ap()
    wsT_d = nc.dram_tensor("wsT", [NB, OSH], F32, kind="ExternalInput").ap()
    wzT_d = nc.dram_tensor("wzT", [NB, OSH], F32, kind="ExternalInput").ap()
    cst_d = nc.dram_tensor("cst", [128, NCST], F32, kind="ExternalInput").ap()
    out_dt = BF16 if OUT_BF16 else F32
    out_d = nc.dram_tensor("out", [OSH, B], out_dt, kind="ExternalOutput").ap()

    with tile.TileContext(nc) as tc, ExitStack() as ctx:
        const = ctx.enter_context(tc.tile_pool(name="const", bufs=1))
        big = ctx.enter_context(tc.tile_pool(name="big", bufs=1))
        xrp = ctx.enter_context(tc.tile_pool(name="xr", bufs=5))
        q8p = ctx.enter_context(tc.tile_pool(name="q8", bufs=3))
        wtp = ctx.enter_context(tc.tile_pool(name="wt", bufs=4))
        wsp = ctx.enter_context(tc.tile_pool(name="wsp", bufs=3))
        wzp = ctx.enter_context(tc.tile_pool(name="wzp", bufs=3))
        rwsp = ctx.enter_context(tc.tile_pool(name="rws", bufs=3))
        tdp = ctx.enter_context(tc.tile_pool(name="td", bufs=4))
        q8wp = ctx.enter_context(tc.tile_pool(name="q8w", bufs=3))
        outp = ctx.enter_context(tc.tile_pool(name="outp", bufs=8))
        psm = ctx.enter_context(tc.tile_pool(name="psm", bufs=1, space="PSUM"))

        # ---- first strip's big DMAs before anything small: fill the pipe ----
        wt0 = wtp.tile([128, OSH], F32, tag="wt")
        nc.sync.dma_start(wt0[:], wT_d[0:128, :])
        xr0 = xrp.tile([128, B], F32, tag="xr")
        nc.sync.dma_start(xr0[:], xT_d[0:128, :])

        # ---- constants: asc | az | bias in one DMA ----
        cst_t = const.tile([128, NCST], F32)
        nc.sync.dma_start(cst_t[:], cst_d)
        asc_t = cst_t[:, 0:NOCT]
        az_t = cst_t[:, NOCT:2 * NOCT]
        bias_t = cst_t[:, 2 * NOCT:]
        ras_t = const.tile([128, NOCT], F32)
        nc.vector.reciprocal(ras_t[:], asc_t)
        # nzsa = -(za * sa)
        nzsa_t = const.tile([128, NOCT], F32)
        nc.vector.scalar_tensor_tensor(nzsa_t[:], az_t, -1.0, asc_t,
                                       OP.mult, OP.mult)
        ident = None
        if DUMMY_STRIP is not None:
            ident = const.tile([128, 128], BF16)
            make_identity(nc, ident[:])

        # ---- resident big tensors ----
        qx_t = big.tile([128, NKT * B], BF16)     # dequant activations
        wq_t = big.tile([128, NKT * OSH], BF16)   # dequant transposed weights

        # 8 psum accumulators: (ot, b-half), each (128, 512) = one bank
        pacc = [psm.tile([128, 512], F32, name=f"pacc{j}") for j in range(8)]

        dummy_emitted = [DUMMY_STRIP is None]

        def emit_scales(oct_):
            ws_t = wsp.tile([128, OSH], F32, tag="ws")
            nc.sync.dma_start(ws_t[:], wsT_d[128 * oct_:128 * (oct_ + 1), :])
            wz_t = wzp.tile([128, OSH], F32, tag="wz")
            nc.sync.dma_start(wz_t[:], wzT_d[128 * oct_:128 * (oct_ + 1), :])
            rws_t = rwsp.tile([128, OSH], F32, tag="rws")
            nc.vector.reciprocal_approx_fast(rws_t[:], ws_t[:])
            return ws_t, wz_t, rws_t

        def emit_strip(i, oct_, r, scales, wx0=None, halves=False):
            kt = r * NOCT + oct_
            ws_t, wz_t, rws_t = scales
            # --- DMAs (w first: its chain is one hop longer) ---
            if wx0 is not None:
                wt_i, xr_i = wx0
            else:
                wt_i = wtp.tile([128, OSH], F32, tag="wt")
                nc.sync.dma_start(wt_i[:], wT_d[128 * kt:128 * (kt + 1), :])
                xr_i = xrp.tile([128, B], F32, tag="xr")
                nc.sync.dma_start(xr_i[:], xT_d[128 * kt:128 * (kt + 1), :])
            # --- W chain: t = w*rws (Pool); q8w = u8(t+wz); d = q8w-wz;
            #     wq = bf16(d*ws) --- (half-split on the last strip so the
            # tail-chain latency after the final DMA is ~halved)
            t_t = tdp.tile([128, OSH], F32, tag="t")
            q8w = q8wp.tile([128, OSH], U8, tag="q8w")
            d_t = tdp.tile([128, OSH], F32, tag="d")
            q8_i = q8p.tile([128, B], U8, tag="q8")
            nh = 2 if halves else 1
            for h in range(nh):
                wsl = slice(h * (OSH // nh), (h + 1) * (OSH // nh))
                nc.gpsimd.tensor_tensor(t_t[:, wsl], wt_i[:, wsl],
                                        rws_t[:, wsl], OP.mult)
                nc.vector.tensor_tensor(q8w[:, wsl], t_t[:, wsl],
                                        wz_t[:, wsl], OP.add)
                nc.vector.tensor_tensor(d_t[:, wsl], q8w[:, wsl],
                                        wz_t[:, wsl], OP.subtract)
                wq_v = wq_t[:, kt * OSH:(kt + 1) * OSH]
                nc.vector.tensor_tensor(wq_v[:, wsl], d_t[:, wsl],
                                        ws_t[:, wsl], OP.mult)
            # --- x chain on ACT: q8 = u8(x*(1/sa)+za); qx = bf16(q8*sa-za*sa)
            for h in range(nh):
                xsl = slice(h * (B // nh), (h + 1) * (B // nh))
                nc.scalar.activation(q8_i[:, xsl], xr_i[:, xsl], AF.Identity,
                                     bias=az_t[:, oct_:oct_ + 1],
                                     scale=ras_t[:, oct_:oct_ + 1])
                qx_v = qx_t[:, kt * B:(kt + 1) * B]
                nc.scalar.activation(qx_v[:, xsl], q8_i[:, xsl], AF.Identity,
                                     bias=nzsa_t[:, oct_:oct_ + 1],
                                     scale=asc_t[:, oct_:oct_ + 1])

        def emit_mms(i, kt):
            if not dummy_emitted[0]:
                dk = DUMMY_STRIP
                dkt = (dk % 4) * NOCT + dk // 4
                nc.tensor.transpose(pacc[0][:, 0:64].bitcast(BF16),
                                    qx_t[:, dkt * B:dkt * B + 128],
                                    ident[:])
                dummy_emitted[0] = True
            for ot in range(NOT):
                lhsT = wq_t[:, kt * OSH + 128 * ot:kt * OSH + 128 * (ot + 1)]
                for b2 in range(2):
                    rhs = qx_t[:, kt * B + 512 * b2:kt * B + 512 * (b2 + 1)]
                    nc.tensor.matmul(pacc[ot * 2 + b2][:], lhsT, rhs,
                                     start=(i == 0), stop=(i == NKT - 1))

        sc = emit_scales(0)
        for oct_ in range(NOCT):
            cur = sc
            for r in range(4):
                i = oct_ * 4 + r
                emit_strip(i, oct_, r, cur,
                           wx0=(wt0, xr0) if i == 0 else None,
                           halves=(i == NKT - 1))
                if r == 0 and oct_ + 1 < NOCT:
                    sc = emit_scales(oct_ + 1)
                if DUMMY_STRIP is None or i >= DUMMY_STRIP:
                    for j in (range(i + 1) if (DUMMY_STRIP is not None
                                               and i == DUMMY_STRIP)
                              else (i,)):
                        emit_mms(j, (j % 4) * NOCT + j // 4)

        # ---- drain: bias add alternating ACT/DVE, out DMA per half ----
        for ot in range(NOT):
            for b2 in range(2):
                j = ot * 2 + b2
                ob = outp.tile([128, 512], out_dt, tag="ob")
                if j % 2 == 0:
                    nc.scalar.activation(ob[:], pacc[j][:], AF.Identity,
                                         bias=bias_t[:, ot:ot + 1], scale=1.0)
                else:
                    nc.vector.tensor_scalar(ob[:], pacc[j][:],
                                            bias_t[:, ot:ot + 1], None, OP.add)
                nc.sync.dma_start(
                    out_d[128 * ot:128 * (ot + 1), 512 * b2:512 * (b2 + 1)],
                    ob[:])

    nc.compile()
    return nc


def _get_nc():
    if "nc" not in _CACHE:
        _CACHE["nc"] = _build_nc()
    return _CACHE["nc"]


def _prep_inputs(x, weight, bias, w_scales, w_zeros, a_scales, a_zeros):
    """Host-side shard/layout prep. Pure slicing/permutation, no arithmetic."""
    x = np.ascontiguousarray(x, np.float32)
    # xT[r*NB + kb, b] = x[b, kb*BS + r]
    xT = np.ascontiguousarray(
        x.reshape(B, NB, BS).transpose(2, 1, 0).reshape(IN, B))
    asc2 = np.asarray(a_scales, np.float32).reshape(NOCT, 128).T
    az2 = np.asarray(a_zeros, np.float32).reshape(NOCT, 128).T
    in_maps = []
    for c in range(NCORES):
        sl = slice(c * OSH, (c + 1) * OSH)
        wsh = np.asarray(weight[sl], np.float32)
        # wT[r*NB + kb, o] = W[o, kb*BS + r]
        wT = np.ascontiguousarray(
            wsh.reshape(OSH, NB, BS).transpose(2, 1, 0).reshape(IN, OSH))
        cst = np.concatenate(
            [asc2, az2,
             np.asarray(bias[sl], np.float32).reshape(NOT, 128).T], axis=1)
        in_maps.append({
            "xT": xT,
            "wT": wT,
            "wsT": np.ascontiguousarray(
                np.asarray(w_scales[sl], np.float32).T),
            "wzT": np.ascontiguousarray(
                np.asarray(w_zeros[sl], np.float32).T),
            "cst": np.ascontiguousarray(cst),
        })
    return in_maps


def kernel(x, weight, bias, w_scales, w_zeros, a_scales, a_zeros, _res_out=None):
    nc = _get_nc()
    in_maps = _prep_inputs(x, weight, bias, w_scales, w_zeros, a_scales, a_zeros)
    res = run_bass_kernel_spmd(nc, in_maps, core_ids=list(range(NCORES)))
    if _res_out is not None:
        _res_out.append(res)
    outT = np.concatenate([np.asarray(res.results[c]["out"], np.float32)
                           for c in range(NCORES)], axis=0)
    return np.ascontiguousarray(outT.T)
